# revision 28
# baseline (speedup 1.0000x reference)
"""MoE (B=2,T=2048,D=768,E=8,K=2,H=1536) Trainium2 kernel.

Sparse expert-parallel over the 8 NeuronCores: the host computes the gate
(softmax + top-2) in numpy, gathers the tokens routed to each expert, and
core e runs expert e's FFN only on its ~B*T*K/E gathered tokens. The
per-token gate weight is applied on device; the host scatter-adds the two
weighted expert outputs per token.

Activations stay feature-major (x^T [D, tok]) so gate/up banks [D,H] and
the down bank [H,D] are already in the stationary-operand (lhsT) layout the
PE wants — no transposes on device. The big GEMMs run in bfloat16: the PE
matmul rate matches float32r (1 col/cycle at N>=256) but bf16 weights get
the compiler-automatic Fast Weight Load path (fp32r LDWEIGHTS ran ~175ns,
pacing the whole MM stream) and all DMA traffic halves. The per-token gate
weight keeps full precision via a two-term bf16 hi+lo split.
"""

import numpy as np
import ml_dtypes

import concourse.bass as bass
import concourse.mybir as mybir
import concourse.tile as tile
from concourse import bass_utils

# Problem shape (hardcoded per contract).
B, T, D, E, H, KTOP = 2, 2048, 768, 8, 1536, 2
NTOK = B * T            # 4096 tokens
TOK = 512               # max tokens per block
DC = D // 128           # 6 chunks of the D (contraction) dim
HC = H // 128           # 12 chunks of the H dim
F32 = mybir.dt.float32
BF16 = mybir.dt.bfloat16
NPBF = ml_dtypes.bfloat16


def _install_axon_ntff_hook():
    """Best-effort: register the antenv.axon_hooks NTFF profile hook that the
    agent image lacks, so trace=True (or BASS_TRACE=1) can profile under axon.
    Never raises."""
    try:
        import sys, types, contextlib, ctypes  # noqa: PLC0415
        import antenv  # noqa: PLC0415
        if "antenv.axon_hooks" in sys.modules:
            return
        _HOOK = [None]
        mod = types.ModuleType("antenv.axon_hooks")
        mod.set_axon_ntff_profile_hook = lambda h: _HOOK.__setitem__(0, h)
        mod.get_axon_ntff_profile_hook = lambda: _HOOK[0]
        sys.modules["antenv.axon_hooks"] = mod
        antenv.axon_hooks = mod

        lib = ctypes.CDLL("/opt/axon/libaxon_pjrt.so")
        if not hasattr(lib, "axon_start_nrt_profile"):
            return
        lib.axon_start_nrt_profile.argtypes = [
            ctypes.POINTER(ctypes.c_int64), ctypes.c_size_t]
        lib.axon_start_nrt_profile.restype = ctypes.c_int64
        lib.axon_stop_nrt_profile.argtypes = [ctypes.c_char_p]
        lib.axon_stop_nrt_profile.restype = ctypes.c_int64

        @contextlib.contextmanager
        def _hook(output_dir, device_ids):
            import jax  # noqa: PLC0415
            jax.devices()
            if device_ids:
                ids = (ctypes.c_int64 * len(device_ids))(*device_ids)
                rc = lib.axon_start_nrt_profile(ids, len(device_ids))
            else:
                rc = lib.axon_start_nrt_profile(None, 0)
            if rc != 0:
                raise RuntimeError(f"axon_start_nrt_profile rc={rc}")
            try:
                yield
            finally:
                lib.axon_stop_nrt_profile(str(output_dir).encode())

        mod.set_axon_ntff_profile_hook(_hook)
    except Exception:
        pass


def _split_multiwaits(nc):
    """This walrus build only supports one sync-wait per instruction; move
    extra waits onto preceding NOPs on the same engine."""
    for fn in nc.m.functions:
        for bb in fn.blocks:
            out = []
            for ins in bb.instructions:
                si = ins.sync_info
                if si is not None and si.on_wait is not None and len(si.on_wait) > 1:
                    waits = list(si.on_wait)
                    for i, w in enumerate(waits[:-1]):
                        out.append(mybir.InstNoOp(
                            name=f"{ins.name}-sw{i}",
                            engine=ins.engine,
                            sync_info=mybir.SyncInfo(on_wait=[w], on_update=[]),
                        ))
                    si.on_wait = [waits[-1]]
                    ins.sync_info = si
                out.append(ins)
            bb.instructions = out
    return nc


def build_nc(npad):
    """Expert FFN on `npad` gathered tokens (feature-major, f32r GEMMs)."""
    # Equal-ish blocks of at most TOK tokens (multiples of 128): balanced
    # blocks beat [512, 512, remainder] because per-block matmul count is
    # fixed while per-matmul cost scales with N.
    ntile = npad // 128
    nblk = -(-ntile // (TOK // 128))
    sizes = [(ntile // nblk + (1 if i < ntile % nblk else 0)) * 128
             for i in range(nblk)]
    blocks = []
    off = 0
    for s in sizes:
        blocks.append((off, s))
        off += s

    # All inputs are host-pre-swizzled to the exact SBUF layout so every DMA
    # is fully contiguous per partition (multi-KB lines -> peak DMA BW):
    #   xg_s[p, off*DC + c*tb + t] = x^T[c*128+p, off+t]       (block-major)
    #   gb_s[p, (ht*DC + c)*128 + j] = gate_bank[c*128+p, ht*128+j]
    #   db_s[p, (hk*DC + c)*128 + j] = down_bank[hk*128+p, c*128+j]
    # Output y_s uses the same block-major layout as xg_s.
    nc = bass.Bass()
    xg_s = nc.dram_tensor("xg_s", [128, DC * npad], BF16, kind="ExternalInput")
    gb_s = nc.dram_tensor("gb_s", [128, DC * H], BF16, kind="ExternalInput")
    ub_s = nc.dram_tensor("ub_s", [128, DC * H], BF16, kind="ExternalInput")
    db_s = nc.dram_tensor("db_s", [128, HC * D], BF16, kind="ExternalInput")
    # Per-token gate weight, host-replicated across the 128 partitions so
    # the on-device broadcast matmul goes away entirely.
    wful = nc.dram_tensor("wful", [128, npad], F32, kind="ExternalInput")
    y_s = nc.dram_tensor("y_s", [128, DC * npad], BF16, kind="ExternalOutput")

    with tile.TileContext(nc) as tc:
        with (
            tc.tile_pool(name="wts", bufs=1) as wts,
            tc.tile_pool(name="xp", bufs=1) as xp,
            tc.tile_pool(name="hp", bufs=24) as hp,
            tc.tile_pool(name="sap", bufs=2) as sap,
            tc.tile_pool(name="yp", bufs=3) as yp,
            tc.tile_pool(name="ps", bufs=8, space="PSUM") as ps,
        ):
            # DMA plan. Both HWDGE rings run in parallel for the startup
            # working set: the ACT ring carries the weight slices ht0-5
            # (dispatched before any SILU is emitted, so the ACT queue is
            # clear again by the first activation), the SP ring carries the
            # x blocks (block 0 split in two so the first matmul can start
            # on the k0-2 half) and later the output chunks. The SWDGE
            # (gpsimd) queue streams the bank bulk (ht6-11 + down + gate
            # row); its transfers run CONCURRENTLY with no priority, so
            # they are dep-chained one-after-another, gated on the first
            # weight slice.
            HTW = DC * 128                     # swizzled width of one ht slice
            NH2 = HC // 2
            gb0s = wts.tile([128, DC, 128], BF16)
            ub0s = wts.tile([128, DC, 128], BF16)
            gb12 = wts.tile([128, 2, DC, 128], BF16)
            ub12 = wts.tile([128, 2, DC, 128], BF16)
            gbA = wts.tile([128, NH2 - 3, DC, 128], BF16)
            ubA = wts.tile([128, NH2 - 3, DC, 128], BF16)
            gbB = wts.tile([128, NH2, DC, 128], BF16)
            ubB = wts.tile([128, NH2, DC, 128], BF16)
            db_h = [wts.tile([128, NH2, DC, 128], BF16, tag=f"dbh{i}",
                             name=f"dbh{i}") for i in range(2)]
            w_sb = wts.tile([128, npad], F32)

            xbs = [None] * nblk

            def emit_xb_dma(bi, ring, split=1):
                off, tb = blocks[bi]
                xbt = xp.tile([128, DC, tb], BF16,
                              tag=f"xb{bi}", name=f"xb{bi}")
                step = DC // split
                for s in range(split):
                    ring.dma_start(
                        xbt[:, s * step:(s + 1) * step, :],
                        xg_s[:, off * DC + s * step * tb:
                             off * DC + (s + 1) * step * tb])
                xbs[bi] = xbt

            # ACT HWDGE ring: gate slices ht0-2, in consumption order.
            nc.scalar.dma_start(gb0s[:], gb_s[:, 0:HTW])
            nc.scalar.dma_start(gb12[:], gb_s[:, HTW:3 * HTW])
            # SP HWDGE ring: x blocks (0 and 1 split for an earlier start).
            emit_xb_dma(0, nc.sync, split=2)
            if nblk > 1:
                emit_xb_dma(1, nc.sync, split=2)
            if nblk > 2:
                emit_xb_dma(2, nc.sync)
            # Everything else on the SWDGE queue, chained one-after-another
            # in consumption order (SWDGE transfers otherwise run
            # CONCURRENTLY with no priority, which would make every tile
            # land only when ALL of them land). The up slices lead the
            # chain: the u-chains trail the a-chains by ~1us each.
            links = [
                [nc.gpsimd.dma_start(ub0s[:], ub_s[:, 0:HTW])],
                [nc.gpsimd.dma_start(ub12[:], ub_s[:, HTW:3 * HTW])],
                [nc.gpsimd.dma_start(gbA[:], gb_s[:, 3 * HTW:NH2 * HTW])],
                [nc.gpsimd.dma_start(ubA[:], ub_s[:, 3 * HTW:NH2 * HTW])],
                [nc.gpsimd.dma_start(gbB[:, 0:3], gb_s[:, NH2 * HTW:
                                                        (NH2 + 3) * HTW]),
                 nc.gpsimd.dma_start(ubB[:, 0:3], ub_s[:, NH2 * HTW:
                                                        (NH2 + 3) * HTW])],
                [nc.gpsimd.dma_start(gbB[:, 3:NH2], gb_s[:, (NH2 + 3) * HTW:
                                                          HC * HTW]),
                 nc.gpsimd.dma_start(ubB[:, 3:NH2], ub_s[:, (NH2 + 3) * HTW:
                                                          HC * HTW])],
                [nc.gpsimd.dma_start(db_h[0][:], db_s[:, 0:NH2 * HTW]),
                 nc.gpsimd.dma_start(db_h[1][:], db_s[:, NH2 * HTW:HC * HTW]),
                 nc.gpsimd.dma_start(w_sb[:], wful[:])],
            ]
            for i in range(1, len(links)):
                for a in links[i]:
                    bass._add_dep_helper(a.ins, links[i - 1][0].ins,
                                         sync=True, reason="SWDGE chain")

            def gb_slice(ht, k):
                if ht == 0:
                    return gb0s[:, k, :]
                if ht < 3:
                    return gb12[:, ht - 1, k, :]
                if ht < NH2:
                    return gbA[:, ht - 3, k, :]
                return gbB[:, ht - NH2, k, :]

            def ub_slice(ht, k):
                if ht == 0:
                    return ub0s[:, k, :]
                if ht < 3:
                    return ub12[:, ht - 1, k, :]
                if ht < NH2:
                    return ubA[:, ht - 3, k, :]
                return ubB[:, ht - NH2, k, :]

            # PE warm-up: short dummy matmuls on memset tiles run while the
            # first weight/x DMAs are still in flight, pulling the Tensor
            # engine out of its low p-state before the real stream starts.
            # Small N so the in-order PE finishes the tail almost instantly
            # once the real operands land.
            wu_w = wts.tile([128, 128], BF16)
            wu_x = wts.tile([128, 128], BF16)
            nc.vector.memset(wu_w[:], 0.0)
            nc.vector.memset(wu_x[:], 0.0)
            wu_ps = ps.tile([128, 128], F32, tag="ps")
            for i in range(24):
                nc.tensor.matmul(wu_ps[:], wu_w[:], wu_x[:],
                                 start=(i == 0), stop=(i == 23))

            hts_all = {b: [] for b in range(nblk)}

            def emit_g1(bis):
                # h = silu(x@gb) * (x@ub), feature-major [H, tb].
                # Interleaving the blocks per ht-slice halves the weight
                # streaming demand rate (each slice feeds 2 blocks' MMs
                # back-to-back), so the DMA never paces the PE.
                for ht in range(HC):
                    for bi in bis:
                        xb = xbs[bi]
                        tb = blocks[bi][1]
                        a_ps = ps.tile([128, tb], F32, tag="ps")
                        for k in range(DC):
                            nc.tensor.matmul(a_ps[:], gb_slice(ht, k),
                                             xb[:, k, :],
                                             start=(k == 0),
                                             stop=(k == DC - 1))
                        u_ps = ps.tile([128, tb], F32, tag="ps")
                        for k in range(DC):
                            nc.tensor.matmul(u_ps[:], ub_slice(ht, k),
                                             xb[:, k, :],
                                             start=(k == 0),
                                             stop=(k == DC - 1))
                        sa = sap.tile([128, tb], F32)
                        nc.scalar.activation(
                            sa[:], a_ps[:],
                            mybir.ActivationFunctionType.Silu)
                        hch = hp.tile([128, tb], BF16, tag="h")
                        nc.vector.tensor_mul(hch[:], sa[:], u_ps[:])
                        hts_all[bi].append(hch)

            def emit_g2(bi):
                # y^T = db^T @ h, scaled by the per-token gate weight.
                off, tb = blocks[bi]
                hts = hts_all.pop(bi)
                for dt in range(DC):
                    y_ps = ps.tile([128, tb], F32, tag="ps")
                    for hk in range(HC):
                        half, kk = divmod(hk, NH2)
                        nc.tensor.matmul(y_ps[:], db_h[half][:, kk, dt, :],
                                         hts[hk][:],
                                         start=(hk == 0), stop=(hk == HC - 1))
                    ysb = yp.tile([128, tb], BF16, tag="ysb")
                    nc.vector.tensor_mul(ysb[:], y_ps[:],
                                         w_sb[:, off:off + tb])
                    # Alternate the output chunks over both HWDGE rings so
                    # the tail never queues behind a backlog.
                    ring = nc.sync if dt % 2 == 0 else nc.scalar
                    ring.dma_start(
                        y_s[:, off * DC + dt * tb:
                            off * DC + (dt + 1) * tb], ysb[:])

            emit_g1(list(range(min(2, nblk))))
            emit_g2(0)
            if nblk > 2:
                emit_g1([2])
            for b in range(1, nblk):
                emit_g2(b)

    return _split_multiwaits(nc)


_NC_CACHE = {}


def _routing(x2d, gate_w):
    """Replicates the reference gate: softmax over E, top-2, renormalize."""
    logits = x2d @ gate_w.T                                  # [NTOK, E] f32
    lmax = logits.max(-1, keepdims=True)
    p = np.exp(logits - lmax)
    p = p / p.sum(-1, keepdims=True)
    idx = np.argsort(-p, axis=-1, kind="stable")[:, :KTOP]   # [NTOK, 2]
    sel = np.take_along_axis(p, idx, -1)
    w = sel / (sel.sum(-1, keepdims=True) + 1e-8)            # [NTOK, 2]
    return idx, w.astype(np.float32)


def kernel(x, gate_w, gate_bank, up_bank, down_bank, _trace=False):
    _install_axon_ntff_hook()
    x = np.asarray(x, dtype=np.float32)
    gate_w = np.asarray(gate_w, dtype=np.float32)
    x2d = np.ascontiguousarray(x.reshape(NTOK, D))

    idx, w = _routing(x2d, gate_w)

    # Token lists per expert.
    tok_idx = []
    tok_w = []
    for e in range(E):
        hit = (idx == e)                        # [NTOK, 2]
        rows = np.nonzero(hit.any(-1))[0]
        tok_idx.append(rows)
        tok_w.append(w[rows, np.argmax(hit[rows], axis=-1)])
    nmax = max(len(r) for r in tok_idx)
    npad = ((nmax + 127) // 128) * 128

    key = npad
    if key not in _NC_CACHE:
        _NC_CACHE[key] = build_nc(npad)
    nc = _NC_CACHE[key]

    ntile = npad // 128
    nblk = -(-ntile // (TOK // 128))
    sizes = [(ntile // nblk + (1 if i < ntile % nblk else 0)) * 128
             for i in range(nblk)]
    offs = list(np.cumsum([0] + sizes[:-1]))

    def swz_x(xgT):
        out = np.empty((128, DC * npad), NPBF)
        for off, tb in zip(offs, sizes):
            seg = xgT[:, off:off + tb].reshape(DC, 128, tb)
            out[:, off * DC:(off + tb) * DC] = \
                seg.transpose(1, 0, 2).reshape(128, DC * tb)
        return out

    in_maps = []
    for e in range(E):
        rows = tok_idx[e]
        xg = np.zeros((npad, D), NPBF)
        xg[: len(rows)] = x2d[rows].astype(NPBF)
        wf = np.zeros(npad, np.float32)
        wf[: len(rows)] = tok_w[e]
        gbs = np.asarray(gate_bank[e], np.float32).astype(NPBF) \
            .reshape(DC, 128, HC, 128) \
            .transpose(1, 2, 0, 3).reshape(128, HC * DC * 128)
        ubs = np.asarray(up_bank[e], np.float32).astype(NPBF) \
            .reshape(DC, 128, HC, 128) \
            .transpose(1, 2, 0, 3).reshape(128, HC * DC * 128)
        dbs = np.asarray(down_bank[e], np.float32).astype(NPBF) \
            .reshape(HC, 128, DC, 128) \
            .transpose(1, 0, 2, 3).reshape(128, HC * DC * 128)
        in_maps.append({
            "xg_s": swz_x(np.ascontiguousarray(xg.T)),
            "gb_s": np.ascontiguousarray(gbs),
            "ub_s": np.ascontiguousarray(ubs),
            "db_s": np.ascontiguousarray(dbs),
            "wful": np.ascontiguousarray(
                np.broadcast_to(wf, (128, npad))),
        })

    res = bass_utils.run_bass_kernel_spmd(
        nc, in_maps, core_ids=list(range(8)), trace=_trace)

    y = np.zeros((NTOK, D), np.float32)
    for e in range(E):
        rows = tok_idx[e]
        ys = np.asarray(res.results[e]["y_s"]).astype(np.float32)
        ygT = np.empty((D, npad), np.float32)
        for off, tb in zip(offs, sizes):
            ygT[:, off:off + tb] = ys[:, off * DC:(off + tb) * DC] \
                .reshape(128, DC, tb).transpose(1, 0, 2).reshape(D, tb)
        y[rows] += ygT[:, : len(rows)].T
    y = y.reshape(B, T, D)
    if _trace:
        return y, res
    return y



# revision 30
# speedup vs baseline: 1.1470x; 1.1470x over previous
"""MoE (B=2,T=2048,D=768,E=8,K=2,H=1536) Trainium2 kernel.

Sparse expert-parallel over the 8 NeuronCores: the host computes the gate
(softmax + top-2) in numpy, gathers the tokens routed to each expert, and
core e runs expert e's FFN only on its ~B*T*K/E gathered tokens. The
per-token gate weight is applied on device; the host scatter-adds the two
weighted expert outputs per token.

Activations stay feature-major (x^T [D, tok]) so gate/up banks [D,H] and
the down bank [H,D] are already in the stationary-operand (lhsT) layout the
PE wants — no transposes on device. The big GEMMs run in bfloat16: the PE
matmul rate matches float32r (1 col/cycle at N>=256) but bf16 weights get
the compiler-automatic Fast Weight Load path (fp32r LDWEIGHTS ran ~175ns,
pacing the whole MM stream) and all DMA traffic halves. The per-token gate
weight keeps full precision via a two-term bf16 hi+lo split.
"""

import numpy as np
import ml_dtypes

import concourse.bass as bass
import concourse.mybir as mybir
import concourse.tile as tile
from concourse import bass_utils

# Problem shape (hardcoded per contract).
B, T, D, E, H, KTOP = 2, 2048, 768, 8, 1536, 2
NTOK = B * T            # 4096 tokens
TOK = 512               # max tokens per block
DC = D // 128           # 6 chunks of the D (contraction) dim
HC = H // 128           # 12 chunks of the H dim
F32 = mybir.dt.float32
BF16 = mybir.dt.bfloat16
NPBF = ml_dtypes.bfloat16


def _install_axon_ntff_hook():
    """Best-effort: register the antenv.axon_hooks NTFF profile hook that the
    agent image lacks, so trace=True (or BASS_TRACE=1) can profile under axon.
    Never raises."""
    try:
        import sys, types, contextlib, ctypes  # noqa: PLC0415
        import antenv  # noqa: PLC0415
        if "antenv.axon_hooks" in sys.modules:
            return
        _HOOK = [None]
        mod = types.ModuleType("antenv.axon_hooks")
        mod.set_axon_ntff_profile_hook = lambda h: _HOOK.__setitem__(0, h)
        mod.get_axon_ntff_profile_hook = lambda: _HOOK[0]
        sys.modules["antenv.axon_hooks"] = mod
        antenv.axon_hooks = mod

        lib = ctypes.CDLL("/opt/axon/libaxon_pjrt.so")
        if not hasattr(lib, "axon_start_nrt_profile"):
            return
        lib.axon_start_nrt_profile.argtypes = [
            ctypes.POINTER(ctypes.c_int64), ctypes.c_size_t]
        lib.axon_start_nrt_profile.restype = ctypes.c_int64
        lib.axon_stop_nrt_profile.argtypes = [ctypes.c_char_p]
        lib.axon_stop_nrt_profile.restype = ctypes.c_int64

        @contextlib.contextmanager
        def _hook(output_dir, device_ids):
            import jax  # noqa: PLC0415
            jax.devices()
            if device_ids:
                ids = (ctypes.c_int64 * len(device_ids))(*device_ids)
                rc = lib.axon_start_nrt_profile(ids, len(device_ids))
            else:
                rc = lib.axon_start_nrt_profile(None, 0)
            if rc != 0:
                raise RuntimeError(f"axon_start_nrt_profile rc={rc}")
            try:
                yield
            finally:
                lib.axon_stop_nrt_profile(str(output_dir).encode())

        mod.set_axon_ntff_profile_hook(_hook)
    except Exception:
        pass


def _split_multiwaits(nc):
    """This walrus build only supports one sync-wait per instruction; move
    extra waits onto preceding NOPs on the same engine."""
    for fn in nc.m.functions:
        for bb in fn.blocks:
            out = []
            for ins in bb.instructions:
                si = ins.sync_info
                if si is not None and si.on_wait is not None and len(si.on_wait) > 1:
                    waits = list(si.on_wait)
                    for i, w in enumerate(waits[:-1]):
                        out.append(mybir.InstNoOp(
                            name=f"{ins.name}-sw{i}",
                            engine=ins.engine,
                            sync_info=mybir.SyncInfo(on_wait=[w], on_update=[]),
                        ))
                    si.on_wait = [waits[-1]]
                    ins.sync_info = si
                out.append(ins)
            bb.instructions = out
    return nc


def build_nc(npad):
    """Expert FFN on `npad` gathered tokens (feature-major, f32r GEMMs)."""
    # Equal-ish blocks of at most TOK tokens (multiples of 128): balanced
    # blocks beat [512, 512, remainder] because per-block matmul count is
    # fixed while per-matmul cost scales with N.
    ntile = npad // 128
    nblk = -(-ntile // (TOK // 128))
    sizes = [(ntile // nblk + (1 if i < ntile % nblk else 0)) * 128
             for i in range(nblk)]
    blocks = []
    off = 0
    for s in sizes:
        blocks.append((off, s))
        off += s

    # All inputs are host-pre-swizzled to the exact SBUF layout so every DMA
    # is fully contiguous per partition (multi-KB lines -> peak DMA BW):
    #   xg_s[p, off*DC + c*tb + t] = x^T[c*128+p, off+t]       (block-major)
    #   gb_s[p, (ht*DC + c)*128 + j] = gate_bank[c*128+p, ht*128+j]
    #   db_s[p, (hk*DC + c)*128 + j] = down_bank[hk*128+p, c*128+j]
    # Output y_s uses the same block-major layout as xg_s.
    nc = bass.Bass()
    xg_s = nc.dram_tensor("xg_s", [128, DC * npad], BF16, kind="ExternalInput")
    gb_s = nc.dram_tensor("gb_s", [128, DC * H], BF16, kind="ExternalInput")
    ub_s = nc.dram_tensor("ub_s", [128, DC * H], BF16, kind="ExternalInput")
    db_s = nc.dram_tensor("db_s", [128, HC * D], BF16, kind="ExternalInput")
    # Per-token gate weight, host-replicated across the 128 partitions so
    # the on-device broadcast matmul goes away entirely.
    wful = nc.dram_tensor("wful", [128, npad], F32, kind="ExternalInput")
    y_s = nc.dram_tensor("y_s", [128, DC * npad], BF16, kind="ExternalOutput")

    with tile.TileContext(nc) as tc:
        with (
            tc.tile_pool(name="wts", bufs=1) as wts,
            tc.tile_pool(name="xp", bufs=1) as xp,
            tc.tile_pool(name="hp", bufs=24) as hp,
            tc.tile_pool(name="sap", bufs=2) as sap,
            tc.tile_pool(name="yp", bufs=3) as yp,
            tc.tile_pool(name="ps", bufs=8, space="PSUM") as ps,
        ):
            # DMA plan. Both HWDGE rings run in parallel for the startup
            # working set: the ACT ring carries the weight slices ht0-5
            # (dispatched before any SILU is emitted, so the ACT queue is
            # clear again by the first activation), the SP ring carries the
            # x blocks (block 0 split in two so the first matmul can start
            # on the k0-2 half) and later the output chunks. The SWDGE
            # (gpsimd) queue streams the bank bulk (ht6-11 + down + gate
            # row); its transfers run CONCURRENTLY with no priority, so
            # they are dep-chained one-after-another, gated on the first
            # weight slice.
            HTW = DC * 128                     # swizzled width of one ht slice
            NH2 = HC // 2
            gb0s = wts.tile([128, DC, 128], BF16)
            ub0s = wts.tile([128, DC, 128], BF16)
            gb12 = wts.tile([128, 2, DC, 128], BF16)
            ub12 = wts.tile([128, 2, DC, 128], BF16)
            gbA = wts.tile([128, NH2 - 3, DC, 128], BF16)
            ubA = wts.tile([128, NH2 - 3, DC, 128], BF16)
            gbB = wts.tile([128, NH2, DC, 128], BF16)
            ubB = wts.tile([128, NH2, DC, 128], BF16)
            db_h = [wts.tile([128, NH2, DC, 128], BF16, tag=f"dbh{i}",
                             name=f"dbh{i}") for i in range(2)]
            w_sb = wts.tile([128, npad], F32)

            xbs = [None] * nblk

            def emit_xb_dma(bi, ring, split=1):
                off, tb = blocks[bi]
                xbt = xp.tile([128, DC, tb], BF16,
                              tag=f"xb{bi}", name=f"xb{bi}")
                step = DC // split
                for s in range(split):
                    ring.dma_start(
                        xbt[:, s * step:(s + 1) * step, :],
                        xg_s[:, off * DC + s * step * tb:
                             off * DC + (s + 1) * step * tb])
                xbs[bi] = xbt

            # ACT HWDGE ring: gate/up slices ht0-2, one transfer per slice,
            # in exact consumption order (a-chain then u-chain per ht).
            d_gb0s = nc.scalar.dma_start(gb0s[:], gb_s[:, 0:HTW])
            nc.scalar.dma_start(ub0s[:], ub_s[:, 0:HTW])
            nc.scalar.dma_start(gb12[:, 0], gb_s[:, HTW:2 * HTW])
            nc.scalar.dma_start(ub12[:, 0], ub_s[:, HTW:2 * HTW])
            nc.scalar.dma_start(gb12[:, 1], gb_s[:, 2 * HTW:3 * HTW])
            nc.scalar.dma_start(ub12[:, 1], ub_s[:, 2 * HTW:3 * HTW])
            # SP HWDGE ring: x blocks (0 and 1 split for an earlier start).
            emit_xb_dma(0, nc.sync, split=2)
            if nblk > 1:
                emit_xb_dma(1, nc.sync, split=2)
            if nblk > 2:
                emit_xb_dma(2, nc.sync)
            # Bulk weights on the SWDGE queue, chained one-after-another
            # (SWDGE transfers otherwise run CONCURRENTLY with no priority,
            # which would make every tile land only when ALL of them land).
            # The chain head waits for the first weight slice.
            links = [
                [nc.gpsimd.dma_start(gbA[:], gb_s[:, 3 * HTW:NH2 * HTW])],
                [nc.gpsimd.dma_start(ubA[:], ub_s[:, 3 * HTW:NH2 * HTW])],
                [nc.gpsimd.dma_start(gbB[:, 0:3], gb_s[:, NH2 * HTW:
                                                        (NH2 + 3) * HTW]),
                 nc.gpsimd.dma_start(ubB[:, 0:3], ub_s[:, NH2 * HTW:
                                                        (NH2 + 3) * HTW])],
                [nc.gpsimd.dma_start(gbB[:, 3:NH2], gb_s[:, (NH2 + 3) * HTW:
                                                          HC * HTW]),
                 nc.gpsimd.dma_start(ubB[:, 3:NH2], ub_s[:, (NH2 + 3) * HTW:
                                                          HC * HTW])],
                [nc.gpsimd.dma_start(db_h[0][:], db_s[:, 0:NH2 * HTW]),
                 nc.gpsimd.dma_start(db_h[1][:], db_s[:, NH2 * HTW:HC * HTW]),
                 nc.gpsimd.dma_start(w_sb[:], wful[:])],
            ]
            for a in links[0]:
                bass._add_dep_helper(a.ins, d_gb0s.ins, sync=True,
                                     reason="SWDGE flood waits for first set")
            for i in range(1, len(links)):
                for a in links[i]:
                    bass._add_dep_helper(a.ins, links[i - 1][0].ins,
                                         sync=True, reason="SWDGE chain")

            def gb_slice(ht, k):
                if ht == 0:
                    return gb0s[:, k, :]
                if ht < 3:
                    return gb12[:, ht - 1, k, :]
                if ht < NH2:
                    return gbA[:, ht - 3, k, :]
                return gbB[:, ht - NH2, k, :]

            def ub_slice(ht, k):
                if ht == 0:
                    return ub0s[:, k, :]
                if ht < 3:
                    return ub12[:, ht - 1, k, :]
                if ht < NH2:
                    return ubA[:, ht - 3, k, :]
                return ubB[:, ht - NH2, k, :]

            # PE warm-up: short dummy matmuls on memset tiles run while the
            # first weight/x DMAs are still in flight, pulling the Tensor
            # engine out of its low p-state before the real stream starts.
            # Small N so the in-order PE finishes the tail almost instantly
            # once the real operands land.
            wu_w = wts.tile([128, 128], BF16)
            wu_x = wts.tile([128, 128], BF16)
            nc.vector.memset(wu_w[:], 0.0)
            nc.vector.memset(wu_x[:], 0.0)
            wu_ps = ps.tile([128, 128], F32, tag="ps")
            for i in range(24):
                nc.tensor.matmul(wu_ps[:], wu_w[:], wu_x[:],
                                 start=(i == 0), stop=(i == 23))

            hts_all = {b: [] for b in range(nblk)}

            def emit_g1(bis):
                # h = silu(x@gb) * (x@ub), feature-major [H, tb].
                # Interleaving the blocks per ht-slice halves the weight
                # streaming demand rate (each slice feeds 2 blocks' MMs
                # back-to-back), so the DMA never paces the PE.
                for ht in range(HC):
                    for bi in bis:
                        xb = xbs[bi]
                        tb = blocks[bi][1]
                        a_ps = ps.tile([128, tb], F32, tag="ps")
                        for k in range(DC):
                            nc.tensor.matmul(a_ps[:], gb_slice(ht, k),
                                             xb[:, k, :],
                                             start=(k == 0),
                                             stop=(k == DC - 1))
                        u_ps = ps.tile([128, tb], F32, tag="ps")
                        for k in range(DC):
                            nc.tensor.matmul(u_ps[:], ub_slice(ht, k),
                                             xb[:, k, :],
                                             start=(k == 0),
                                             stop=(k == DC - 1))
                        sa = sap.tile([128, tb], F32)
                        nc.scalar.activation(
                            sa[:], a_ps[:],
                            mybir.ActivationFunctionType.Silu)
                        hch = hp.tile([128, tb], BF16, tag="h")
                        nc.vector.tensor_mul(hch[:], sa[:], u_ps[:])
                        hts_all[bi].append(hch)

            def emit_g2(bi):
                # y^T = db^T @ h, scaled by the per-token gate weight.
                off, tb = blocks[bi]
                hts = hts_all.pop(bi)
                for dt in range(DC):
                    y_ps = ps.tile([128, tb], F32, tag="ps")
                    for hk in range(HC):
                        half, kk = divmod(hk, NH2)
                        nc.tensor.matmul(y_ps[:], db_h[half][:, kk, dt, :],
                                         hts[hk][:],
                                         start=(hk == 0), stop=(hk == HC - 1))
                    ysb = yp.tile([128, tb], BF16, tag="ysb")
                    nc.vector.tensor_mul(ysb[:], y_ps[:],
                                         w_sb[:, off:off + tb])
                    # Alternate the output chunks over both HWDGE rings so
                    # the tail never queues behind a backlog.
                    ring = nc.sync if dt % 2 == 0 else nc.scalar
                    ring.dma_start(
                        y_s[:, off * DC + dt * tb:
                            off * DC + (dt + 1) * tb], ysb[:])

            emit_g1(list(range(min(2, nblk))))
            emit_g2(0)
            if nblk > 2:
                emit_g1([2])
            for b in range(1, nblk):
                emit_g2(b)

    return _split_multiwaits(nc)


_NC_CACHE = {}


def _routing(x2d, gate_w):
    """Replicates the reference gate: softmax over E, top-2, renormalize."""
    logits = x2d @ gate_w.T                                  # [NTOK, E] f32
    lmax = logits.max(-1, keepdims=True)
    p = np.exp(logits - lmax)
    p = p / p.sum(-1, keepdims=True)
    idx = np.argsort(-p, axis=-1, kind="stable")[:, :KTOP]   # [NTOK, 2]
    sel = np.take_along_axis(p, idx, -1)
    w = sel / (sel.sum(-1, keepdims=True) + 1e-8)            # [NTOK, 2]
    return idx, w.astype(np.float32)


def kernel(x, gate_w, gate_bank, up_bank, down_bank, _trace=False):
    _install_axon_ntff_hook()
    x = np.asarray(x, dtype=np.float32)
    gate_w = np.asarray(gate_w, dtype=np.float32)
    x2d = np.ascontiguousarray(x.reshape(NTOK, D))

    idx, w = _routing(x2d, gate_w)

    # Token lists per expert.
    tok_idx = []
    tok_w = []
    for e in range(E):
        hit = (idx == e)                        # [NTOK, 2]
        rows = np.nonzero(hit.any(-1))[0]
        tok_idx.append(rows)
        tok_w.append(w[rows, np.argmax(hit[rows], axis=-1)])
    nmax = max(len(r) for r in tok_idx)
    npad = ((nmax + 127) // 128) * 128

    key = npad
    if key not in _NC_CACHE:
        _NC_CACHE[key] = build_nc(npad)
    nc = _NC_CACHE[key]

    ntile = npad // 128
    nblk = -(-ntile // (TOK // 128))
    sizes = [(ntile // nblk + (1 if i < ntile % nblk else 0)) * 128
             for i in range(nblk)]
    offs = list(np.cumsum([0] + sizes[:-1]))

    def swz_x(xgT):
        out = np.empty((128, DC * npad), NPBF)
        for off, tb in zip(offs, sizes):
            seg = xgT[:, off:off + tb].reshape(DC, 128, tb)
            out[:, off * DC:(off + tb) * DC] = \
                seg.transpose(1, 0, 2).reshape(128, DC * tb)
        return out

    in_maps = []
    for e in range(E):
        rows = tok_idx[e]
        xg = np.zeros((npad, D), NPBF)
        xg[: len(rows)] = x2d[rows].astype(NPBF)
        wf = np.zeros(npad, np.float32)
        wf[: len(rows)] = tok_w[e]
        gbs = np.asarray(gate_bank[e], np.float32).astype(NPBF) \
            .reshape(DC, 128, HC, 128) \
            .transpose(1, 2, 0, 3).reshape(128, HC * DC * 128)
        ubs = np.asarray(up_bank[e], np.float32).astype(NPBF) \
            .reshape(DC, 128, HC, 128) \
            .transpose(1, 2, 0, 3).reshape(128, HC * DC * 128)
        dbs = np.asarray(down_bank[e], np.float32).astype(NPBF) \
            .reshape(HC, 128, DC, 128) \
            .transpose(1, 0, 2, 3).reshape(128, HC * DC * 128)
        in_maps.append({
            "xg_s": swz_x(np.ascontiguousarray(xg.T)),
            "gb_s": np.ascontiguousarray(gbs),
            "ub_s": np.ascontiguousarray(ubs),
            "db_s": np.ascontiguousarray(dbs),
            "wful": np.ascontiguousarray(
                np.broadcast_to(wf, (128, npad))),
        })

    res = bass_utils.run_bass_kernel_spmd(
        nc, in_maps, core_ids=list(range(8)), trace=_trace)

    y = np.zeros((NTOK, D), np.float32)
    for e in range(E):
        rows = tok_idx[e]
        ys = np.asarray(res.results[e]["y_s"]).astype(np.float32)
        ygT = np.empty((D, npad), np.float32)
        for off, tb in zip(offs, sizes):
            ygT[:, off:off + tb] = ys[:, off * DC:(off + tb) * DC] \
                .reshape(128, DC, tb).transpose(1, 0, 2).reshape(D, tb)
        y[rows] += ygT[:, : len(rows)].T
    y = y.reshape(B, T, D)
    if _trace:
        return y, res
    return y



# revision 34
# speedup vs baseline: 1.1582x; 1.0098x over previous
"""MoE (B=2,T=2048,D=768,E=8,K=2,H=1536) Trainium2 kernel.

Sparse expert-parallel over the 8 NeuronCores: the host computes the gate
(softmax + top-2) in numpy, gathers the tokens routed to each expert, and
core e runs expert e's FFN only on its ~B*T*K/E gathered tokens. The
per-token gate weight is applied on device; the host scatter-adds the two
weighted expert outputs per token.

Activations stay feature-major (x^T [D, tok]) so gate/up banks [D,H] and
the down bank [H,D] are already in the stationary-operand (lhsT) layout the
PE wants — no transposes on device. The big GEMMs run in bfloat16: the PE
matmul rate matches float32r (1 col/cycle at N>=256) but bf16 weights get
the compiler-automatic Fast Weight Load path (fp32r LDWEIGHTS ran ~175ns,
pacing the whole MM stream) and all DMA traffic halves. The per-token gate
weight keeps full precision via a two-term bf16 hi+lo split.
"""

import numpy as np
import ml_dtypes

import concourse.bass as bass
import concourse.mybir as mybir
import concourse.tile as tile
from concourse import bass_utils

# Problem shape (hardcoded per contract).
B, T, D, E, H, KTOP = 2, 2048, 768, 8, 1536, 2
NTOK = B * T            # 4096 tokens
TOK = 512               # max tokens per block
DC = D // 128           # 6 chunks of the D (contraction) dim
HC = H // 128           # 12 chunks of the H dim
F32 = mybir.dt.float32
BF16 = mybir.dt.bfloat16
NPBF = ml_dtypes.bfloat16


def _install_axon_ntff_hook():
    """Best-effort: register the antenv.axon_hooks NTFF profile hook that the
    agent image lacks, so trace=True (or BASS_TRACE=1) can profile under axon.
    Never raises."""
    try:
        import sys, types, contextlib, ctypes  # noqa: PLC0415
        import antenv  # noqa: PLC0415
        if "antenv.axon_hooks" in sys.modules:
            return
        _HOOK = [None]
        mod = types.ModuleType("antenv.axon_hooks")
        mod.set_axon_ntff_profile_hook = lambda h: _HOOK.__setitem__(0, h)
        mod.get_axon_ntff_profile_hook = lambda: _HOOK[0]
        sys.modules["antenv.axon_hooks"] = mod
        antenv.axon_hooks = mod

        lib = ctypes.CDLL("/opt/axon/libaxon_pjrt.so")
        if not hasattr(lib, "axon_start_nrt_profile"):
            return
        lib.axon_start_nrt_profile.argtypes = [
            ctypes.POINTER(ctypes.c_int64), ctypes.c_size_t]
        lib.axon_start_nrt_profile.restype = ctypes.c_int64
        lib.axon_stop_nrt_profile.argtypes = [ctypes.c_char_p]
        lib.axon_stop_nrt_profile.restype = ctypes.c_int64

        @contextlib.contextmanager
        def _hook(output_dir, device_ids):
            import jax  # noqa: PLC0415
            jax.devices()
            if device_ids:
                ids = (ctypes.c_int64 * len(device_ids))(*device_ids)
                rc = lib.axon_start_nrt_profile(ids, len(device_ids))
            else:
                rc = lib.axon_start_nrt_profile(None, 0)
            if rc != 0:
                raise RuntimeError(f"axon_start_nrt_profile rc={rc}")
            try:
                yield
            finally:
                lib.axon_stop_nrt_profile(str(output_dir).encode())

        mod.set_axon_ntff_profile_hook(_hook)
    except Exception:
        pass


def _split_multiwaits(nc):
    """This walrus build only supports one sync-wait per instruction; move
    extra waits onto preceding NOPs on the same engine."""
    for fn in nc.m.functions:
        for bb in fn.blocks:
            out = []
            for ins in bb.instructions:
                si = ins.sync_info
                if si is not None and si.on_wait is not None and len(si.on_wait) > 1:
                    waits = list(si.on_wait)
                    for i, w in enumerate(waits[:-1]):
                        out.append(mybir.InstNoOp(
                            name=f"{ins.name}-sw{i}",
                            engine=ins.engine,
                            sync_info=mybir.SyncInfo(on_wait=[w], on_update=[]),
                        ))
                    si.on_wait = [waits[-1]]
                    ins.sync_info = si
                out.append(ins)
            bb.instructions = out
    return nc


def build_nc(npad):
    """Expert FFN on `npad` gathered tokens (feature-major, f32r GEMMs)."""
    # Equal-ish blocks of at most TOK tokens (multiples of 128): balanced
    # blocks beat [512, 512, remainder] because per-block matmul count is
    # fixed while per-matmul cost scales with N.
    ntile = npad // 128
    nblk = -(-ntile // (TOK // 128))
    sizes = [(ntile // nblk + (1 if i < ntile % nblk else 0)) * 128
             for i in range(nblk)]
    blocks = []
    off = 0
    for s in sizes:
        blocks.append((off, s))
        off += s

    # All inputs are host-pre-swizzled to the exact SBUF layout so every DMA
    # is fully contiguous per partition (multi-KB lines -> peak DMA BW):
    #   xg_s[p, off*DC + c*tb + t] = x^T[c*128+p, off+t]       (block-major)
    #   gb_s[p, (ht*DC + c)*128 + j] = gate_bank[c*128+p, ht*128+j]
    #   db_s[p, (hk*DC + c)*128 + j] = down_bank[hk*128+p, c*128+j]
    # Output y_s uses the same block-major layout as xg_s.
    nc = bass.Bass()
    xg_s = nc.dram_tensor("xg_s", [128, DC * npad], BF16, kind="ExternalInput")
    gb_s = nc.dram_tensor("gb_s", [128, DC * H], BF16, kind="ExternalInput")
    ub_s = nc.dram_tensor("ub_s", [128, DC * H], BF16, kind="ExternalInput")
    db_s = nc.dram_tensor("db_s", [128, HC * D], BF16, kind="ExternalInput")
    # Per-token gate weight, host-replicated across the 128 partitions so
    # the on-device broadcast matmul goes away entirely.
    wful = nc.dram_tensor("wful", [128, npad], F32, kind="ExternalInput")
    y_s = nc.dram_tensor("y_s", [128, DC * npad], BF16, kind="ExternalOutput")

    with tile.TileContext(nc) as tc:
        with (
            tc.tile_pool(name="wts", bufs=1) as wts,
            tc.tile_pool(name="xp", bufs=1) as xp,
            tc.tile_pool(name="hp", bufs=24) as hp,
            tc.tile_pool(name="sap", bufs=2) as sap,
            tc.tile_pool(name="yp", bufs=3) as yp,
            tc.tile_pool(name="ps", bufs=8, space="PSUM") as ps,
        ):
            # DMA plan. Both HWDGE rings run in parallel for the startup
            # working set: the ACT ring carries the weight slices ht0-5
            # (dispatched before any SILU is emitted, so the ACT queue is
            # clear again by the first activation), the SP ring carries the
            # x blocks (block 0 split in two so the first matmul can start
            # on the k0-2 half) and later the output chunks. The SWDGE
            # (gpsimd) queue streams the bank bulk (ht6-11 + down + gate
            # row); its transfers run CONCURRENTLY with no priority, so
            # they are dep-chained one-after-another, gated on the first
            # weight slice.
            HTW = DC * 128                     # swizzled width of one ht slice
            NH2 = HC // 2
            gb0s = wts.tile([128, DC, 128], BF16)
            ub0s = wts.tile([128, DC, 128], BF16)
            gb12 = wts.tile([128, 2, DC, 128], BF16)
            ub12 = wts.tile([128, 2, DC, 128], BF16)
            gbA = wts.tile([128, NH2 - 3, DC, 128], BF16)
            ubA = wts.tile([128, NH2 - 3, DC, 128], BF16)
            gbB = wts.tile([128, NH2, DC, 128], BF16)
            ubB = wts.tile([128, NH2, DC, 128], BF16)
            db_h = [wts.tile([128, NH2, DC, 128], BF16, tag=f"dbh{i}",
                             name=f"dbh{i}") for i in range(2)]
            w_sb = wts.tile([128, npad], F32)

            xbs = [None] * nblk

            def emit_xb_dma(bi, ring, split=1):
                off, tb = blocks[bi]
                xbt = xp.tile([128, DC, tb], BF16,
                              tag=f"xb{bi}", name=f"xb{bi}")
                step = DC // split
                for s in range(split):
                    ring.dma_start(
                        xbt[:, s * step:(s + 1) * step, :],
                        xg_s[:, off * DC + s * step * tb:
                             off * DC + (s + 1) * step * tb])
                xbs[bi] = xbt

            # ACT HWDGE ring: gate/up slices ht0-2, one transfer per slice,
            # in exact consumption order (a-chain then u-chain per ht).
            d_gb0s = nc.scalar.dma_start(gb0s[:], gb_s[:, 0:HTW])
            nc.scalar.dma_start(ub0s[:], ub_s[:, 0:HTW])
            nc.scalar.dma_start(gb12[:, 0], gb_s[:, HTW:2 * HTW])
            nc.scalar.dma_start(ub12[:, 0], ub_s[:, HTW:2 * HTW])
            nc.scalar.dma_start(gb12[:, 1], gb_s[:, 2 * HTW:3 * HTW])
            nc.scalar.dma_start(ub12[:, 1], ub_s[:, 2 * HTW:3 * HTW])
            # SP HWDGE ring: x blocks (0 and 1 split for an earlier start).
            emit_xb_dma(0, nc.sync, split=2)
            if nblk > 1:
                emit_xb_dma(1, nc.sync, split=2)
            if nblk > 2:
                emit_xb_dma(2, nc.sync)
            # Bulk weights on the SWDGE queue, chained one-after-another
            # (SWDGE transfers otherwise run CONCURRENTLY with no priority,
            # which would make every tile land only when ALL of them land).
            # The chain head waits for the first weight slice.
            links = [
                [nc.gpsimd.dma_start(gbA[:], gb_s[:, 3 * HTW:NH2 * HTW])],
                [nc.gpsimd.dma_start(ubA[:], ub_s[:, 3 * HTW:NH2 * HTW])],
                [nc.gpsimd.dma_start(gbB[:, 0:3], gb_s[:, NH2 * HTW:
                                                        (NH2 + 3) * HTW]),
                 nc.gpsimd.dma_start(ubB[:, 0:3], ub_s[:, NH2 * HTW:
                                                        (NH2 + 3) * HTW])],
                [nc.gpsimd.dma_start(gbB[:, 3:NH2], gb_s[:, (NH2 + 3) * HTW:
                                                          HC * HTW]),
                 nc.gpsimd.dma_start(ubB[:, 3:NH2], ub_s[:, (NH2 + 3) * HTW:
                                                          HC * HTW])],
                [nc.gpsimd.dma_start(db_h[0][:], db_s[:, 0:NH2 * HTW]),
                 nc.gpsimd.dma_start(db_h[1][:], db_s[:, NH2 * HTW:HC * HTW]),
                 nc.gpsimd.dma_start(w_sb[:], wful[:])],
            ]
            for a in links[0]:
                bass._add_dep_helper(a.ins, d_gb0s.ins, sync=True,
                                     reason="SWDGE flood waits for first set")
            for i in range(1, len(links)):
                for a in links[i]:
                    bass._add_dep_helper(a.ins, links[i - 1][0].ins,
                                         sync=True, reason="SWDGE chain")

            def gb_slice(ht, k):
                if ht == 0:
                    return gb0s[:, k, :]
                if ht < 3:
                    return gb12[:, ht - 1, k, :]
                if ht < NH2:
                    return gbA[:, ht - 3, k, :]
                return gbB[:, ht - NH2, k, :]

            def ub_slice(ht, k):
                if ht == 0:
                    return ub0s[:, k, :]
                if ht < 3:
                    return ub12[:, ht - 1, k, :]
                if ht < NH2:
                    return ubA[:, ht - 3, k, :]
                return ubB[:, ht - NH2, k, :]

            # PE warm-up: short dummy matmuls on memset tiles run while the
            # first weight/x DMAs are still in flight, pulling the Tensor
            # engine out of its low p-state before the real stream starts.
            # Small N so the in-order PE finishes the tail almost instantly
            # once the real operands land.
            wu_w = wts.tile([128, 128], BF16)
            wu_x = wts.tile([128, 128], BF16)
            nc.vector.memset(wu_w[:], 0.0)
            nc.vector.memset(wu_x[:], 0.0)
            wu_ps = ps.tile([128, 128], F32, tag="ps")
            for i in range(16):
                nc.tensor.matmul(wu_ps[:], wu_w[:], wu_x[:],
                                 start=(i == 0), stop=(i == 15))

            hts_all = {b: [None] * HC for b in range(nblk)}

            def emit_g1(bis, hts=range(HC)):
                # h = silu(x@gb) * (x@ub), feature-major [H, tb].
                # Interleaving the blocks per ht-slice halves the weight
                # streaming demand rate (each slice feeds 2 blocks' MMs
                # back-to-back), so the DMA never paces the PE.
                for ht in hts:
                    for bi in bis:
                        xb = xbs[bi]
                        tb = blocks[bi][1]
                        a_ps = ps.tile([128, tb], F32, tag="ps")
                        for k in range(DC):
                            nc.tensor.matmul(a_ps[:], gb_slice(ht, k),
                                             xb[:, k, :],
                                             start=(k == 0),
                                             stop=(k == DC - 1))
                        u_ps = ps.tile([128, tb], F32, tag="ps")
                        for k in range(DC):
                            nc.tensor.matmul(u_ps[:], ub_slice(ht, k),
                                             xb[:, k, :],
                                             start=(k == 0),
                                             stop=(k == DC - 1))
                        sa = sap.tile([128, tb], F32)
                        nc.scalar.activation(
                            sa[:], a_ps[:],
                            mybir.ActivationFunctionType.Silu)
                        hch = hp.tile([128, tb], BF16, tag="h")
                        nc.vector.tensor_mul(hch[:], sa[:], u_ps[:])
                        hts_all[bi][ht] = hch

            def emit_g2(bi):
                # y^T = db^T @ h, scaled by the per-token gate weight.
                off, tb = blocks[bi]
                hts = hts_all.pop(bi)
                for dt in range(DC):
                    y_ps = ps.tile([128, tb], F32, tag="ps")
                    for hk in range(HC):
                        half, kk = divmod(hk, NH2)
                        nc.tensor.matmul(y_ps[:], db_h[half][:, kk, dt, :],
                                         hts[hk][:],
                                         start=(hk == 0), stop=(hk == HC - 1))
                    ysb = yp.tile([128, tb], BF16, tag="ysb")
                    nc.vector.tensor_mul(ysb[:], y_ps[:],
                                         w_sb[:, off:off + tb])
                    # Alternate the output chunks over both HWDGE rings so
                    # the tail never queues behind a backlog.
                    ring = nc.sync if dt % 2 == 0 else nc.scalar
                    ring.dma_start(
                        y_s[:, off * DC + dt * tb:
                            off * DC + (dt + 1) * tb], ysb[:])

            # Block 0 runs ht0-1 solo while block 1's x is still landing
            # (halves the startup DMA demand); the pair then interleaves
            # ht2-11, and block 1 catches up on ht0-1 at the end with all
            # weights already resident.
            if nblk > 1:
                emit_g1([0], hts=range(2))
                emit_g1([0, 1], hts=range(2, HC))
                emit_g1([1], hts=range(2))
            else:
                emit_g1([0])
            emit_g2(0)
            if nblk > 2:
                emit_g1([2])
            for b in range(1, nblk):
                emit_g2(b)

    return _split_multiwaits(nc)


_NC_CACHE = {}


def _routing(x2d, gate_w):
    """Replicates the reference gate: softmax over E, top-2, renormalize."""
    logits = x2d @ gate_w.T                                  # [NTOK, E] f32
    lmax = logits.max(-1, keepdims=True)
    p = np.exp(logits - lmax)
    p = p / p.sum(-1, keepdims=True)
    idx = np.argsort(-p, axis=-1, kind="stable")[:, :KTOP]   # [NTOK, 2]
    sel = np.take_along_axis(p, idx, -1)
    w = sel / (sel.sum(-1, keepdims=True) + 1e-8)            # [NTOK, 2]
    return idx, w.astype(np.float32)


def kernel(x, gate_w, gate_bank, up_bank, down_bank, _trace=False):
    _install_axon_ntff_hook()
    x = np.asarray(x, dtype=np.float32)
    gate_w = np.asarray(gate_w, dtype=np.float32)
    x2d = np.ascontiguousarray(x.reshape(NTOK, D))

    idx, w = _routing(x2d, gate_w)

    # Token lists per expert.
    tok_idx = []
    tok_w = []
    for e in range(E):
        hit = (idx == e)                        # [NTOK, 2]
        rows = np.nonzero(hit.any(-1))[0]
        tok_idx.append(rows)
        tok_w.append(w[rows, np.argmax(hit[rows], axis=-1)])
    nmax = max(len(r) for r in tok_idx)
    npad = ((nmax + 127) // 128) * 128

    key = npad
    if key not in _NC_CACHE:
        _NC_CACHE[key] = build_nc(npad)
    nc = _NC_CACHE[key]

    ntile = npad // 128
    nblk = -(-ntile // (TOK // 128))
    sizes = [(ntile // nblk + (1 if i < ntile % nblk else 0)) * 128
             for i in range(nblk)]
    offs = list(np.cumsum([0] + sizes[:-1]))

    def swz_x(xgT):
        out = np.empty((128, DC * npad), NPBF)
        for off, tb in zip(offs, sizes):
            seg = xgT[:, off:off + tb].reshape(DC, 128, tb)
            out[:, off * DC:(off + tb) * DC] = \
                seg.transpose(1, 0, 2).reshape(128, DC * tb)
        return out

    in_maps = []
    for e in range(E):
        rows = tok_idx[e]
        xg = np.zeros((npad, D), NPBF)
        xg[: len(rows)] = x2d[rows].astype(NPBF)
        wf = np.zeros(npad, np.float32)
        wf[: len(rows)] = tok_w[e]
        gbs = np.asarray(gate_bank[e], np.float32).astype(NPBF) \
            .reshape(DC, 128, HC, 128) \
            .transpose(1, 2, 0, 3).reshape(128, HC * DC * 128)
        ubs = np.asarray(up_bank[e], np.float32).astype(NPBF) \
            .reshape(DC, 128, HC, 128) \
            .transpose(1, 2, 0, 3).reshape(128, HC * DC * 128)
        dbs = np.asarray(down_bank[e], np.float32).astype(NPBF) \
            .reshape(HC, 128, DC, 128) \
            .transpose(1, 0, 2, 3).reshape(128, HC * DC * 128)
        in_maps.append({
            "xg_s": swz_x(np.ascontiguousarray(xg.T)),
            "gb_s": np.ascontiguousarray(gbs),
            "ub_s": np.ascontiguousarray(ubs),
            "db_s": np.ascontiguousarray(dbs),
            "wful": np.ascontiguousarray(
                np.broadcast_to(wf, (128, npad))),
        })

    res = bass_utils.run_bass_kernel_spmd(
        nc, in_maps, core_ids=list(range(8)), trace=_trace)

    y = np.zeros((NTOK, D), np.float32)
    for e in range(E):
        rows = tok_idx[e]
        ys = np.asarray(res.results[e]["y_s"]).astype(np.float32)
        ygT = np.empty((D, npad), np.float32)
        for off, tb in zip(offs, sizes):
            ygT[:, off:off + tb] = ys[:, off * DC:(off + tb) * DC] \
                .reshape(128, DC, tb).transpose(1, 0, 2).reshape(D, tb)
        y[rows] += ygT[:, : len(rows)].T
    y = y.reshape(B, T, D)
    if _trace:
        return y, res
    return y

